# revision 4
# baseline (speedup 1.0000x reference)
"""Trainium2 Bass kernel for pairwise-MLP GNN message passing.

dro[b,i,j] = w3^T relu(W2^T relu(hA_i + hB_j) + b2) + b3, with the first
linear layer factorized as hA_i + hB_j (no relu between concat and W1).

Sharding: robot-row dimension N=512 split across 8 cores (64 rows each).

v2 engine split (per robot row i):
  L1 on DVE (bf16, 4x mode): t1_k = relu(hBT_k + hA[:,i])           3 ops
  L2 on PE  (bf16): z2[jt] = t1^T @ W2p, K-tiles {128,128,64},
     the K=64 tails of adjacent jt pairs run concurrently in row
     groups (0,0)/(64,0)  -> 10 x N-cycle slots per i
  L3a on ACT: one merged Relu over all 4 PSUM banks -> r (bf16 SBUF)
  L3b on DVE (bf16 4x): osigP/N[:,i] = sum over pos/neg w3 column
     blocks of r (W2 columns pre-scaled by |w3| and permuted pos|neg)
  epilogue: osb = osigP - osigN -> DMA j-major; host transposes + b3.
"""

import numpy as np
import ml_dtypes

import concourse.bass as bass
import concourse.mybir as mybir
import concourse.tile as tile
from concourse import bacc
from concourse import bass_utils
from concourse.masks import make_identity

F32 = mybir.dt.float32
F32R = mybir.dt.float32r
BF16 = mybir.dt.bfloat16
ALU = mybir.AluOpType
ACTF = mybir.ActivationFunctionType

B, N, E, L = 2, 512, 128, 32
D = E + L            # 160
H = 2 * D            # 320
NCORES = 8
NI = N // NCORES     # 64 robot rows per core
NJT = 4              # j-tiles of 128
MS = [(0, 128), (128, 128), (256, 64)]   # m-tiles of H (hA/hB build)

_CACHE = {}


def _build(hpp, hnp, with_bias):
    """hpp/hnp: even-padded sizes of the pos/neg w3 column blocks."""
    nn = hpp + hnp                    # matmul free dim (W2p column count)
    kc = 65 if with_bias else 64      # third k-tile height (64 data + ones)

    nc = bacc.Bacc("TRN2", target_bir_lowering=False, debug=False,
                   enable_asserts=False, num_devices=NCORES)

    robot = nc.dram_tensor("robot", [B, NI, E], F32, kind="ExternalInput").ap()
    obj = nc.dram_tensor("obj", [B, N, E], F32, kind="ExternalInput").ap()
    W1A = nc.dram_tensor("W1A", [E, H], F32, kind="ExternalInput").ap()
    W1B = nc.dram_tensor("W1B", [E, H], F32, kind="ExternalInput").ap()
    zAT = nc.dram_tensor("zAT", [H, B], F32, kind="ExternalInput").ap()
    zBT = nc.dram_tensor("zBT", [H, B], F32, kind="ExternalInput").ap()
    w2a = nc.dram_tensor("w2a", [128, nn], BF16, kind="ExternalInput").ap()
    w2b = nc.dram_tensor("w2b", [128, nn], BF16, kind="ExternalInput").ap()
    # k rows 256:320 (+ones row if biased); duplicated at partitions 64+
    # in the unbiased case so the K=64 tails can pair in row groups.
    w2c = nc.dram_tensor("w2c", [kc if with_bias else 128, nn], BF16,
                         kind="ExternalInput").ap()
    out = nc.dram_tensor("out", [B, N, NI], F32, kind="ExternalOutput").ap()

    with tile.TileContext(nc) as tc:
        with tc.tile_pool(name="persist", bufs=1) as pp:
            ident = pp.tile([128, 128], F32, tag="ident")
            make_identity(nc, ident[:])
            # force the ACT Relu table load early so it overlaps setup
            warm = pp.tile([1, 1], F32, tag="warm")
            nc.scalar.activation(warm[:], ident[0:1, 0:1], ACTF.Relu)

            # ---- weight tiles (bf16 direct from HBM) ----
            w2at = pp.tile([128, nn], BF16, tag="w2a")
            nc.sync.dma_start(w2at[:], w2a)
            w2bt = pp.tile([128, nn], BF16, tag="w2b")
            nc.scalar.dma_start(w2bt[:], w2b)
            w2ct = pp.tile([kc if with_bias else 128, nn], BF16, tag="w2c")
            nc.sync.dma_start(w2ct[:], w2c)

            # f32r W1 halves for the setup matmuls
            with tc.tile_pool(name="wstg", bufs=2) as wstg:
                stg = wstg.tile([E, H], F32, tag="wstg")
                nc.sync.dma_start(stg[:], W1A)
                w1a = pp.tile([E, H], F32R, tag="w1a")
                nc.vector.tensor_copy(w1a[:], stg[:])
                stg = wstg.tile([E, H], F32, tag="wstg")
                nc.scalar.dma_start(stg[:], W1B)
                w1b = pp.tile([E, H], F32R, tag="w1b")
                nc.vector.tensor_copy(w1b[:], stg[:])

            zat, zbt = [], []
            for m, (m0, sz) in enumerate(MS):
                t = pp.tile([sz, B], F32, tag=f"zat_{m}")
                nc.sync.dma_start(t[:], zAT[m0:m0 + sz, :])
                zat.append(t)
                t = pp.tile([sz, B], F32, tag=f"zbt_{m}")
                nc.sync.dma_start(t[:], zBT[m0:m0 + sz, :])
                zbt.append(t)

            hbt = {}  # (b, k) -> bf16 tile: k<2 [128,N]; k=2 [128,N] dup'd
            hat = {}  # (b, k) -> bf16 tile: k<2 [128,NI]; k=2 [128,NI] dup'd

            # ---- setup: build hA^T, hB^T on device (bf16 outputs) ----
            with tc.tile_pool(name="s_sb", bufs=2) as ssb, \
                 tc.tile_pool(name="s_ps", bufs=2, space="PSUM") as sps:
                for b in range(B):
                    # hB^T[b]: [H, N] = W1B^T @ obj[b]^T (+ zB bias)
                    objT_ps = sps.tile([128, N], F32, tag="objT_ps")
                    for jt in range(NJT):
                        stg = ssb.tile([128, E], F32, tag="stg", bufs=2)
                        qs = ([nc.sync, nc.scalar, nc.sync, nc.scalar]
                              if b == 0 else
                              [nc.gpsimd, nc.gpsimd, nc.gpsimd, nc.gpsimd])
                        qs[jt].dma_start(
                            stg[:], obj[b, jt * 128:(jt + 1) * 128, :])
                        nc.tensor.transpose(objT_ps[:, jt * 128:(jt + 1) * 128],
                                            stg[:], ident[:])
                    objT = ssb.tile([128, N], F32R, tag="objT")
                    nc.vector.tensor_copy(objT[:], objT_ps[:])
                    for m, (m0, sz) in enumerate(MS):
                        hps = sps.tile([sz, N], F32, tag="hps")
                        nc.tensor.matmul(hps[:], w1b[:, m0:m0 + sz], objT[:],
                                         start=True, stop=True)
                        szk = 128 if m < 2 else kc
                        t = pp.tile([128 if m == 2 else szk, N], BF16,
                                    tag=f"hbt_{b}_{m}")
                        nc.vector.tensor_scalar(
                            out=t[0:sz, :], in0=hps[:],
                            scalar1=zbt[m][:, b:b + 1],
                            scalar2=None, op0=ALU.add)
                        if m == 2:
                            if with_bias:
                                nc.gpsimd.memset(t[64:65, :], 1.0)
                            else:
                                # duplicate k rows 256:320 at partitions
                                # 64:128 for the paired K=64 matmul tails
                                nc.gpsimd.dma_start(t[64:128, :], t[0:64, :])
                        hbt[(b, m)] = t

                    # hA^T[b]: [H, NI] from robot[b] @ W1A (+ zA bias)
                    stg2 = ssb.tile([NI, E], F32, tag="stg2")
                    (nc.scalar if b == 0 else nc.gpsimd).dma_start(
                        stg2[:], robot[b, :, :])
                    robT_ps = sps.tile([128, NI], F32, tag="robT_ps")
                    nc.tensor.transpose(robT_ps[:], stg2[:], ident[0:NI, 0:NI])
                    robT = ssb.tile([128, NI], F32R, tag="robT")
                    nc.vector.tensor_copy(robT[:], robT_ps[:])
                    for m, (m0, sz) in enumerate(MS):
                        aps_ = sps.tile([sz, NI], F32, tag="aps")
                        nc.tensor.matmul(aps_[:], w1a[:, m0:m0 + sz], robT[:],
                                         start=True, stop=True)
                        szk = 128 if m < 2 else kc
                        t = pp.tile([128 if m == 2 else szk, NI], F32,
                                    tag=f"hat_{b}_{m}")
                        nc.vector.tensor_scalar(
                            out=t[0:sz, :], in0=aps_[:],
                            scalar1=zat[m][:, b:b + 1],
                            scalar2=None, op0=ALU.add)
                        if m == 2:
                            if with_bias:
                                nc.gpsimd.memset(t[64:65, :], 0.0)
                            else:
                                nc.gpsimd.dma_start(t[64:128, :], t[0:64, :])
                        hat[(b, m)] = t

            # ---- main loop ----
            with tc.tile_pool(name="t1p", bufs=2) as t1p, \
                 tc.tile_pool(name="z2p", bufs=2, space="PSUM") as z2p, \
                 tc.tile_pool(name="rp", bufs=2) as rp, \
                 tc.tile_pool(name="scr", bufs=2) as scr, \
                 tc.tile_pool(name="accp", bufs=2) as accp, \
                 tc.tile_pool(name="outp", bufs=2) as outp:
                for b in range(B):
                    osigP = {jt: accp.tile([128, NI], F32, tag=f"osp_{jt}",
                                           name=f"osp_{jt}_{b}")
                             for jt in range(NJT)}
                    osigN = {jt: accp.tile([128, NI], F32, tag=f"osn_{jt}",
                                           name=f"osn_{jt}_{b}")
                             for jt in range(NJT)}
                    rprev = None

                    for i in range(NI):
                        # L1: t1_k = relu(hBT_k + hA_col) on DVE, bf16 4x
                        t1 = []
                        for k in range(3):
                            kp = 128 if (k < 2 or not with_bias) else kc
                            t = t1p.tile([kp, N], BF16, tag=f"t1_{k}")
                            nc.vector.tensor_scalar(
                                out=t[:], in0=hbt[(b, k)][0:kp, :],
                                scalar1=hat[(b, k)][0:kp, i:i + 1],
                                scalar2=0.0, op0=ALU.add, op1=ALU.max)
                            t1.append(t)
                        # L2: z2[jt] = t1^T[jt] @ W2p on PE (bf16)
                        quad = z2p.tile([128, NJT, 512], F32, tag="quad")
                        for half in range(2):
                            jts = (0, 1) if half == 0 else (2, 3)
                            for jt in jts:
                                js = slice(jt * 128, (jt + 1) * 128)
                                nc.tensor.matmul(
                                    quad[:, jt, 0:nn], t1[0][:, js], w2at[:],
                                    start=True, stop=False)
                                nc.tensor.matmul(
                                    quad[:, jt, 0:nn], t1[1][:, js], w2bt[:],
                                    start=False, stop=False)
                            if with_bias:
                                for jt in jts:
                                    js = slice(jt * 128, (jt + 1) * 128)
                                    nc.tensor.matmul(
                                        quad[:, jt, 0:nn], t1[2][0:kc, js],
                                        w2ct[0:kc, :],
                                        start=False, stop=True)
                            else:
                                # K=64 tails of the jt pair run concurrently
                                # in row groups (0,0) and (64,0)
                                jt0, jt1 = jts
                                js0 = slice(jt0 * 128, (jt0 + 1) * 128)
                                js1 = slice(jt1 * 128, (jt1 + 1) * 128)
                                nc.tensor.matmul(
                                    quad[:, jt0, 0:nn], t1[2][0:64, js0],
                                    w2ct[0:64, :], start=False, stop=True)
                                nc.tensor.matmul(
                                    quad[:, jt1, 0:nn], t1[2][64:128, js1],
                                    w2ct[64:128, :], start=False, stop=True)
                        # L3a: merged Relu over all 4 banks -> bf16 SBUF
                        r = rp.tile([128, NJT, nn], BF16, tag="r")
                        nc.scalar.activation(r[:], quad[:, :, 0:nn], ACTF.Relu)

                        # L3b for the previous i (software pipeline: keeps
                        # DVE fed with L1(i+1) before it blocks on ACT(i))
                        if rprev is not None:
                            self_sums(nc, scr, rprev, osigP, osigN,
                                      i - 1, hpp, hnp, nn)
                        rprev = r

                    self_sums(nc, scr, rprev, osigP, osigN, NI - 1,
                              hpp, hnp, nn)

                    # epilogue: osb = osigP - osigN, store j-major
                    for jt in range(NJT):
                        osb = outp.tile([128, NI], F32, tag=f"osb_{jt % 2}")
                        nc.vector.scalar_tensor_tensor(
                            out=osb[:], in0=osigP[jt][:], scalar=0.0,
                            in1=osigN[jt][:], op0=ALU.add, op1=ALU.subtract)
                        nc.sync.dma_start(
                            out[b, jt * 128:(jt + 1) * 128, :], osb[:])

    nc.compile()
    return nc


def self_sums(nc, scr, r, osigP, osigN, i, hpp, hnp, nn):
    """L3b: per jt, accumulate pos/neg block sums of r into osig columns."""
    for jt in range(4):
        sp = scr.tile([128, nn], BF16, tag="scrP")
        nc.vector.tensor_scalar(
            out=sp[:, 0:hpp], in0=r[:, jt, 0:hpp], scalar1=0.0,
            scalar2=0.0, op0=ALU.add, op1=ALU.add,
            accum_out=osigP[jt][:, i:i + 1])
        sn = scr.tile([128, nn], BF16, tag="scrN")
        nc.vector.tensor_scalar(
            out=sn[:, 0:hnp], in0=r[:, jt, hpp:nn], scalar1=0.0,
            scalar2=0.0, op0=ALU.add, op1=ALU.add,
            accum_out=osigN[jt][:, i:i + 1])


def _prep(robot_embedding_tf, object_embedding_tf, z, W1, b1, W2, b2, W3, b3):
    """Host-side weight prep (O(H^2)) + per-core input maps."""
    f = np.float32
    bf = ml_dtypes.bfloat16
    robot = np.ascontiguousarray(robot_embedding_tf, dtype=f)
    obj = np.ascontiguousarray(object_embedding_tf, dtype=f)
    z = np.asarray(z, dtype=f)
    W1 = np.asarray(W1, dtype=f)
    b1 = np.asarray(b1, dtype=f)
    W2 = np.asarray(W2, dtype=f)
    b2 = np.asarray(b2, dtype=f)
    W3 = np.asarray(W3, dtype=f)
    b3 = np.asarray(b3, dtype=f)

    w3 = W3[:, 0]
    aw3 = np.abs(w3)
    pos = np.nonzero(w3 >= 0)[0]
    neg = np.nonzero(w3 < 0)[0]
    hp, hn = len(pos), len(neg)
    hpp = hp + (hp & 1)
    hnp = hn + (hn & 1)
    nn = hpp + hnp

    with_bias = bool(np.any(b2))
    W2p = W2 * aw3[None, :]
    b2p = b2 * aw3
    kc = 65 if with_bias else 64
    # column-permuted + |w3|-scaled W2 (+ bias row), even-padded blocks
    W2cols = np.zeros((H + 1, nn), dtype=f)
    W2cols[:H, 0:hp] = W2p[:, pos]
    W2cols[:H, hpp:hpp + hn] = W2p[:, neg]
    W2cols[H, 0:hp] = b2p[pos]
    W2cols[H, hpp:hpp + hn] = b2p[neg]

    w2a_ = np.ascontiguousarray(W2cols[0:128], dtype=bf)
    w2b_ = np.ascontiguousarray(W2cols[128:256], dtype=bf)
    if with_bias:
        w2c_ = np.ascontiguousarray(W2cols[256:256 + kc], dtype=bf)
    else:
        w2c_ = np.ascontiguousarray(
            np.concatenate([W2cols[256:320], W2cols[256:320]], axis=0),
            dtype=bf)

    zA = z @ W1[E:D, :]                 # [B, H]
    zB = z @ W1[D + E:, :] + b1[None, :]
    zAT = np.ascontiguousarray(zA.T, dtype=f)
    zBT = np.ascontiguousarray(zB.T, dtype=f)
    W1A = np.ascontiguousarray(W1[0:E, :], dtype=f)
    W1B = np.ascontiguousarray(W1[D:D + E, :], dtype=f)

    shared = dict(obj=obj, W1A=W1A, W1B=W1B, zAT=zAT, zBT=zBT,
                  w2a=w2a_, w2b=w2b_, w2c=w2c_)
    in_maps = []
    for c in range(NCORES):
        m = dict(shared)
        m["robot"] = np.ascontiguousarray(robot[:, c * NI:(c + 1) * NI, :])
        in_maps.append(m)
    return in_maps, (hpp, hnp, with_bias), float(b3[0])


def _run(trace=False, **inputs):
    in_maps, key, b3v = _prep(**inputs)
    if key not in _CACHE:
        _CACHE[key] = _build(*key)
    nc = _CACHE[key]
    res = bass_utils.run_bass_kernel_spmd(
        nc, in_maps, core_ids=list(range(NCORES)), trace=trace)
    dro = np.empty((B, N, N), dtype=np.float32)
    for c in range(NCORES):
        # device output is j-major [B, N, NI]; transpose to [B, NI, N]
        dro[:, c * NI:(c + 1) * NI, :] = np.transpose(
            res.results[c]["out"], (0, 2, 1))
    if b3v != 0.0:
        dro += b3v
    return dro, res


def kernel(**inputs) -> np.ndarray:
    dro, _ = _run(trace=False, **inputs)
    return dro
